# revision 6
# baseline (speedup 1.0000x reference)
"""GCN autoencoder (2x GCNConv + Linear) on 8 Trainium2 NeuronCores.

Strategy (sharding_hint): nodes are sharded across the 8 cores in contiguous
chunks; edges are partitioned by destination node.  Per conv layer, each core
gathers source-node features with dma_gather (rows land one-per-partition),
builds a one-hot "indicator" matrix (dst-column one-hot scaled by the GCN
degree normalization) on the vector engine, and scatter-adds via PE matmuls
accumulating in PSUM per 128-wide destination block.  Dense transforms
(W1, W2, fc) run as regular matmuls with features on partitions and nodes on
the free dim.  The halo exchange of conv2's source features (t2 = h1 @ W2)
is two AllGather collectives (split so the second overlaps conv2 compute).
"""

import math

import numpy as np

import concourse.bass as bass
import concourse.tile as tile
from concourse import bacc, mybir
from concourse.bass_utils import run_bass_kernel_spmd

# ---------------- problem constants (hardcoded per contract) ----------------
N = 50000
E = 500000
D_IN = 128
D_HID = 128  # conv1 out = 2*D_HID = 256
D_OUT = 6
CORES = 8
CHUNK = N // CORES  # 6250

W = 128            # destination-block width (psum tile free dim)
BPG = 7            # dst blocks per gather-call group
SLAB = 512         # node slab for dense transforms
HALF1 = 25000      # conv1 gather-source split (int16 index limit)
H2 = CHUNK // 2    # 3125; conv2 allgather split within each chunk

F32 = mybir.dt.float32
I16 = mybir.dt.int16


def _cd(a, b):
    return -(-a // b)


def _wrap_idx(ix):
    """[L] int -> [128, L//16] int16 wrapped in 16 partitions, replicated x8."""
    n = len(ix)
    arr = np.zeros((16, n // 16), np.int16)
    arr[np.arange(n) % 16, np.arange(n) // 16] = ix.astype(np.int16)
    return np.tile(arr, (8, 1))


def _plan_conv(src, dst, norm, h, idx_vals, n_cores, chunk, w, bpg):
    """Uniform-across-cores edge tiling plan for one conv's propagation.

    Returns dict with tile-count structure plus per-core idx/dstcol/norm
    arrays (padded; pads gather row 0 and never match the indicator).
    """
    nb = _cd(chunk, w)
    g_n = _cd(nb, bpg)
    m = dst // chunk
    dl = dst % chunk
    b = dl // w
    g = b // bpg
    bl = b % bpg

    cnt = np.zeros((n_cores, g_n, 2, bpg), np.int64)
    np.add.at(cnt, (m, g, h, bl), 1)
    t_cell = -(-cnt.max(axis=0) // 128)  # [G,2,BPG]
    t_tot = int(t_cell.sum())

    tile_base = np.zeros((g_n, 2, bpg), np.int64)
    run = 0
    for gg in range(g_n):
        for hh in range(2):
            for bb in range(bpg):
                tile_base[gg, hh, bb] = run
                run += t_cell[gg, hh, bb]

    t_call = t_cell.sum(axis=2)          # [G,2] tiles per gather call
    l_gh = t_call * 128                  # idx count per call
    call_base = np.zeros((g_n, 2), np.int64)
    off16 = np.zeros((g_n, 2), np.int64)
    run_t, run_i = 0, 0
    for gg in range(g_n):
        for hh in range(2):
            call_base[gg, hh] = run_t
            off16[gg, hh] = run_i
            run_t += t_call[gg, hh]
            run_i += l_gh[gg, hh] // 16
    it16 = run_i

    flat_base = tile_base.reshape(-1)
    eap = t_tot * 128
    per_core = []
    for mm in range(n_cores):
        sel = np.nonzero(m == mm)[0]
        key = (g[sel] * 2 + h[sel]) * bpg + bl[sel]
        order = np.argsort(key, kind="stable")
        sel = sel[order]
        key = key[order]
        kcnt = np.bincount(key, minlength=g_n * 2 * bpg)
        starts = np.concatenate([[0], np.cumsum(kcnt)[:-1]])
        rank = np.arange(len(sel)) - starts[key]
        pos = flat_base[key] * 128 + rank

        dc = np.full(eap, -5.0, np.float32)
        nm = np.zeros(eap, np.float32)
        ix = np.zeros(eap, np.int64)
        dc[pos] = (dl[sel] - (dl[sel] // w) * w).astype(np.float32)
        nm[pos] = norm[sel]
        ix[pos] = idx_vals[sel]

        # idx stream wrapped per (g,h) call
        idx_cols = []
        for gg in range(g_n):
            for hh in range(2):
                lo = call_base[gg, hh] * 128
                ln = int(l_gh[gg, hh])
                if ln:
                    idx_cols.append(_wrap_idx(ix[lo:lo + ln]))
        idxw = (np.concatenate(idx_cols, axis=1) if idx_cols
                else np.zeros((128, 1), np.int16))
        per_core.append(dict(
            dc=np.ascontiguousarray(dc.reshape(t_tot, 128).T),
            nm=np.ascontiguousarray(nm.reshape(t_tot, 128).T),
            idx=idxw,
        ))
    return dict(nb=nb, g_n=g_n, t_cell=t_cell, t_tot=t_tot,
                tile_base=tile_base, t_call=t_call, l_gh=l_gh,
                call_base=call_base, off16=off16, it16=max(it16, 16),
                per_core=per_core)


def _build(plan1, plan2, use_bf16):
    DT = mybir.dt.bfloat16 if use_bf16 else F32
    nc = bacc.Bacc("TRN2", target_bir_lowering=False, debug=False,
                   num_devices=CORES, num_swdge_queues=4)

    x_d = nc.dram_tensor("x", [N, D_IN], DT, kind="ExternalInput").ap()
    w1_d = nc.dram_tensor("w1", [D_IN, 2 * D_HID], F32, kind="ExternalInput").ap()
    w2a_d = nc.dram_tensor("w2a", [D_HID, D_HID], F32, kind="ExternalInput").ap()
    w2b_d = nc.dram_tensor("w2b", [D_HID, D_HID], F32, kind="ExternalInput").ap()
    wfc_d = nc.dram_tensor("wfc", [D_HID, D_OUT], F32, kind="ExternalInput").ap()
    b1a_d = nc.dram_tensor("b1a", [D_HID, 1], F32, kind="ExternalInput").ap()
    b1b_d = nc.dram_tensor("b1b", [D_HID, 1], F32, kind="ExternalInput").ap()
    b2_d = nc.dram_tensor("b2", [D_HID, 1], F32, kind="ExternalInput").ap()
    bfc_d = nc.dram_tensor("bfc", [D_OUT, 1], F32, kind="ExternalInput").ap()
    id_d = nc.dram_tensor("ident", [128, 128], DT, kind="ExternalInput").ap()
    idx1_d = nc.dram_tensor("idx1", [128, plan1["it16"]], I16, kind="ExternalInput").ap()
    idx2_d = nc.dram_tensor("idx2", [128, plan2["it16"]], I16, kind="ExternalInput").ap()
    dc1_d = nc.dram_tensor("dc1", [128, plan1["t_tot"]], F32, kind="ExternalInput").ap()
    nm1_d = nc.dram_tensor("nm1", [128, plan1["t_tot"]], F32, kind="ExternalInput").ap()
    dc2_d = nc.dram_tensor("dc2", [128, plan2["t_tot"]], F32, kind="ExternalInput").ap()
    nm2_d = nc.dram_tensor("nm2", [128, plan2["t_tot"]], F32, kind="ExternalInput").ap()
    y_d = nc.dram_tensor("y", [D_OUT, CHUNK], F32, kind="ExternalOutput").ap()

    with tile.TileContext(nc) as tc:
        with (
            tc.tile_pool(name="const", bufs=1) as constp,
            tc.tile_pool(name="meta", bufs=1) as metap,
            tc.tile_pool(name="msgs", bufs=3 if use_bf16 else 2) as msgsp,
            tc.tile_pool(name="ind", bufs=2) as indp,
            tc.tile_pool(name="big", bufs=1) as bigp,
            tc.tile_pool(name="sm", bufs=3) as smp,
            tc.tile_pool(name="ps", bufs=3, space="PSUM") as psp,
            tc.tile_pool(name="pst", bufs=2, space="PSUM") as pstp,
            tc.tile_pool(name="dram", bufs=1, space="DRAM") as dramp,
        ):
            # ---- constants ----
            tmax = int(max(plan1["t_call"].max(), plan2["t_call"].max()))
            iota = constp.tile([128, tmax, W], DT, tag="iota")
            nc.gpsimd.iota(iota[:], pattern=[[0, tmax], [1, W]], base=0,
                           channel_multiplier=0,
                           allow_small_or_imprecise_dtypes=True)
            ident = constp.tile([128, 128], DT, tag="ident")
            nc.sync.dma_start(ident[:], id_d[:])
            w1_sb = constp.tile([D_IN, 2 * D_HID], F32, tag="w1")
            nc.sync.dma_start(w1_sb[:], w1_d[:])
            w2a_sb = constp.tile([D_HID, D_HID], F32, tag="w2a")
            nc.sync.dma_start(w2a_sb[:], w2a_d[:])
            w2b_sb = constp.tile([D_HID, D_HID], F32, tag="w2b")
            nc.sync.dma_start(w2b_sb[:], w2b_d[:])
            wfc_sb = constp.tile([D_HID, D_OUT], F32, tag="wfc")
            nc.sync.dma_start(wfc_sb[:], wfc_d[:])
            b1a_sb = constp.tile([D_HID, 1], F32, tag="b1a")
            nc.sync.dma_start(b1a_sb[:], b1a_d[:])
            b1b_sb = constp.tile([D_HID, 1], F32, tag="b1b")
            nc.sync.dma_start(b1b_sb[:], b1b_d[:])
            b2_sb = constp.tile([D_HID, 1], F32, tag="b2")
            nc.sync.dma_start(b2_sb[:], b2_d[:])
            bfc_sb = constp.tile([D_OUT, 1], F32, tag="bfc")
            nc.sync.dma_start(bfc_sb[:], bfc_d[:])

            # internal DRAM for the halo exchange
            t2a = dramp.tile([H2, D_HID], DT, tag="t2a")
            t2b = dramp.tile([H2, D_HID], DT, tag="t2b")
            ag1 = dramp.tile([CORES * H2, D_HID], DT, tag="ag1")
            ag2 = dramp.tile([CORES * H2, D_HID], DT, tag="ag2")

            qcount = [0]

            def propagation(plan, idx_d_, dc_d_, nm_d_, src_views, src_steps,
                            out_tile, out_relu_bias):
                g_n, nb = plan["g_n"], plan["nb"]
                t_cell, tile_base = plan["t_cell"], plan["tile_base"]
                l_gh, call_base, off16 = plan["l_gh"], plan["call_base"], plan["off16"]

                idx_sb = metap.tile([128, plan["it16"]], I16, tag="idx")
                nc.sync.dma_start(idx_sb[:], idx_d_[:])
                dc_sb = metap.tile([128, plan["t_tot"]], F32, tag="dc")
                nc.sync.dma_start(dc_sb[:], dc_d_[:])
                nm_sb = metap.tile([128, plan["t_tot"]], F32, tag="nm")
                nc.sync.dma_start(nm_sb[:], nm_d_[:])

                for g in range(g_n):
                    msgs = {}
                    inds = {}
                    for h in (0, 1):
                        ln = int(l_gh[g, h])
                        if ln == 0:
                            continue
                        nt_call = ln // 128
                        mt = msgsp.tile([128, nt_call, 128], DT, tag="msgs")
                        nc.gpsimd.dma_gather(
                            mt[:], src_views[h],
                            idx_sb[:, int(off16[g, h]): int(off16[g, h]) + ln // 16],
                            ln, ln, 128, elem_step=src_steps[h],
                            single_packet=False, queue_num=qcount[0] % 4,
                        )
                        qcount[0] += 1
                        msgs[h] = mt
                        # batched indicator build for the whole call
                        cb = int(call_base[g, h])
                        it = indp.tile([128, nt_call, W], DT, tag="ind")
                        dcb = dc_sb[:, cb:cb + nt_call].unsqueeze(2).to_broadcast(
                            [128, nt_call, W])
                        nmb = nm_sb[:, cb:cb + nt_call].unsqueeze(2).to_broadcast(
                            [128, nt_call, W])
                        nc.vector.tensor_tensor(
                            it[:], iota[:, :nt_call, :], dcb,
                            op=mybir.AluOpType.is_equal)
                        nc.vector.tensor_tensor(
                            it[:], it[:], nmb, op=mybir.AluOpType.mult)
                        inds[h] = it
                    for bl in range(BPG):
                        b = g * BPG + bl
                        if b >= nb:
                            break
                        wb = min(W, CHUNK - b * W)
                        n_t = int(t_cell[g, 0, bl] + t_cell[g, 1, bl])
                        if n_t == 0:
                            continue
                        ps = psp.tile([128, W], F32, tag="ps")
                        k = 0
                        for h in (0, 1):
                            tb = int(tile_base[g, h, bl])
                            cb = int(call_base[g, h])
                            for t in range(int(t_cell[g, h, bl])):
                                tl = tb - cb + t     # tile within gather call
                                nc.tensor.matmul(
                                    ps[:], msgs[h][:, tl, :], inds[h][:, tl, :],
                                    start=(k == 0), stop=(k == n_t - 1),
                                )
                                k += 1
                        if out_relu_bias is None:
                            nc.scalar.activation(
                                out_tile[:, b * W: b * W + wb], ps[:, :wb],
                                mybir.ActivationFunctionType.Copy)
                        else:
                            nc.scalar.activation(
                                out_tile[:, b * W: b * W + wb], ps[:, :wb],
                                mybir.ActivationFunctionType.Relu,
                                bias=out_relu_bias[:, 0:1])

            # ---------------- conv1 ----------------
            agg1 = bigp.tile([D_IN, CHUNK], F32, tag="agg")
            propagation(plan1, idx1_d, dc1_d, nm1_d,
                        [x_d[0:N:2, :], x_d[1:N:2, :]], [2 * D_IN, 2 * D_IN],
                        agg1, None)

            # transform1: h1 = relu(W1.T @ agg1 + b1)
            h1a = bigp.tile([D_HID, CHUNK], F32, tag="h1a")
            h1b = bigp.tile([D_HID, CHUNK], F32, tag="h1b")
            for s0 in range(0, CHUNK, SLAB):
                sl = min(SLAB, CHUNK - s0)
                pa = pstp.tile([128, SLAB], F32, tag="pst")
                nc.tensor.matmul(pa[:, :sl], w1_sb[:, 0:D_HID], agg1[:, s0:s0 + sl])
                nc.scalar.activation(h1a[:, s0:s0 + sl], pa[:, :sl],
                                     mybir.ActivationFunctionType.Relu,
                                     bias=b1a_sb[:, 0:1])
                pb = pstp.tile([128, SLAB], F32, tag="pst")
                nc.tensor.matmul(pb[:, :sl], w1_sb[:, D_HID:2 * D_HID],
                                 agg1[:, s0:s0 + sl])
                nc.scalar.activation(h1b[:, s0:s0 + sl], pb[:, :sl],
                                     mybir.ActivationFunctionType.Relu,
                                     bias=b1b_sb[:, 0:1])

            # transform2: t2 = W2.T-halves @ h1 (no bias), split by node parity
            t2te = bigp.tile([D_HID, H2], DT, tag="t2te")
            t2to = bigp.tile([D_HID, H2], DT, tag="t2to")
            for s0 in range(0, CHUNK, SLAB):
                sl = min(SLAB, CHUNK - s0)
                pc = pstp.tile([128, SLAB], F32, tag="pst")
                nc.tensor.matmul(pc[:, :sl], w2a_sb[:], h1a[:, s0:s0 + sl],
                                 start=True, stop=False)
                nc.tensor.matmul(pc[:, :sl], w2b_sb[:], h1b[:, s0:s0 + sl],
                                 start=False, stop=True)
                nc.vector.tensor_copy(t2te[:, s0 // 2: s0 // 2 + (sl + 1) // 2],
                                      pc[:, 0:sl:2])
                nc.vector.tensor_copy(t2to[:, s0 // 2: s0 // 2 + sl // 2],
                                      pc[:, 1:sl:2])

            # transpose to node-major and store each parity half to DRAM
            for t2p, t2d in ((t2te, t2a), (t2to, t2b)):
                for j in range(_cd(H2, 128)):
                    c0 = j * 128
                    cl = min(128, H2 - c0)
                    pt = pstp.tile([128, 128], DT, tag="pst")
                    nc.tensor.transpose(pt[:cl, :], t2p[:, c0:c0 + cl], ident[:])
                    tn = smp.tile([128, 128], DT, tag="tn")
                    nc.vector.tensor_copy(tn[:cl, :], pt[:cl, :])
                    nc.sync.dma_start(t2d[c0:c0 + cl, :], tn[:cl, :])

            # halo exchange: two AllGathers (second overlaps conv2 h=0 work)
            nc.gpsimd.collective_compute(
                "AllGather", mybir.AluOpType.bypass,
                replica_groups=[list(range(CORES))],
                ins=[t2a.opt()], outs=[ag1.opt()])
            nc.gpsimd.collective_compute(
                "AllGather", mybir.AluOpType.bypass,
                replica_groups=[list(range(CORES))],
                ins=[t2b.opt()], outs=[ag2.opt()])

            # ---------------- conv2 ----------------
            out2 = bigp.tile([D_HID, CHUNK], F32, tag="agg")
            propagation(plan2, idx2_d, dc2_d, nm2_d,
                        [ag1[:, :], ag2[:, :]], [D_HID, D_HID], out2, b2_sb)

            # fc: y = Wfc.T @ out2 + bfc
            for s0 in range(0, CHUNK, SLAB):
                sl = min(SLAB, CHUNK - s0)
                pf = pstp.tile([D_OUT, SLAB], F32, tag="pst")
                nc.tensor.matmul(pf[:, :sl], wfc_sb[:], out2[:, s0:s0 + sl])
                yt = smp.tile([D_OUT, SLAB], F32, tag="yt")
                nc.vector.tensor_scalar(yt[:, :sl], pf[:, :sl], bfc_sb[:, 0:1], None,
                                        op0=mybir.AluOpType.add)
                nc.sync.dma_start(y_d[:, s0:s0 + sl], yt[:, :sl])

    nc.compile()
    return nc


def _preprocess(x, W1, b1, W2, b2, Wfc, bfc, edge_index, use_bf16):
    src = np.concatenate([edge_index[0], np.arange(N, dtype=np.int64)])
    dst = np.concatenate([edge_index[1], np.arange(N, dtype=np.int64)])
    deg = np.bincount(dst, minlength=N).astype(np.float32)
    isq = deg.astype(np.float32) ** -0.5
    norm = (isq[src] * isq[dst]).astype(np.float32)

    h1 = (src & 1).astype(np.int64)
    plan1 = _plan_conv(src, dst, norm, h1, src // 2, CORES, CHUNK, W, BPG)

    j = src % CHUNK
    h2 = (j & 1).astype(np.int64)
    idx2 = (src // CHUNK) * H2 + j // 2
    plan2 = _plan_conv(src, dst, norm, h2, idx2, CORES, CHUNK, W, BPG)

    ndt = np.dtype("bfloat16") if use_bf16 else np.float32
    common = dict(
        x=np.ascontiguousarray(x.astype(ndt)),
        w1=np.ascontiguousarray(W1.astype(np.float32)),
        w2a=np.ascontiguousarray(W2[:D_HID].astype(np.float32)),
        w2b=np.ascontiguousarray(W2[D_HID:].astype(np.float32)),
        wfc=np.ascontiguousarray(Wfc.astype(np.float32)),
        b1a=np.ascontiguousarray(b1[:D_HID].reshape(D_HID, 1).astype(np.float32)),
        b1b=np.ascontiguousarray(b1[D_HID:].reshape(D_HID, 1).astype(np.float32)),
        b2=np.ascontiguousarray(b2.reshape(D_HID, 1).astype(np.float32)),
        bfc=np.ascontiguousarray(bfc.reshape(D_OUT, 1).astype(np.float32)),
        ident=np.eye(128, dtype=np.float32).astype(ndt),
    )
    in_maps = []
    for m in range(CORES):
        im = dict(common)
        im["idx1"] = plan1["per_core"][m]["idx"]
        im["dc1"] = plan1["per_core"][m]["dc"]
        im["nm1"] = plan1["per_core"][m]["nm"]
        im["idx2"] = plan2["per_core"][m]["idx"]
        im["dc2"] = plan2["per_core"][m]["dc"]
        im["nm2"] = plan2["per_core"][m]["nm"]
        in_maps.append(im)
    return plan1, plan2, in_maps


_CACHE = {}


def _get_compiled(x, W1, b1, W2, b2, Wfc, bfc, edge_index, use_bf16=True):
    plan1, plan2, in_maps = _preprocess(
        x, W1, b1, W2, b2, Wfc, bfc, edge_index, use_bf16)
    key = ("nc", use_bf16, plan1["t_tot"], plan2["t_tot"])
    if key not in _CACHE:
        _CACHE[key] = _build(plan1, plan2, use_bf16)
    return _CACHE[key], in_maps


def kernel(x, W1, b1, W2, b2, Wfc, bfc, edge_index, use_bf16=True, trace=False):
    x = np.asarray(x)
    edge_index = np.asarray(edge_index).astype(np.int64)
    nc, in_maps = _get_compiled(np.asarray(x), np.asarray(W1), np.asarray(b1),
                                np.asarray(W2), np.asarray(b2), np.asarray(Wfc),
                                np.asarray(bfc), edge_index, use_bf16)
    res = run_bass_kernel_spmd(nc, in_maps, list(range(CORES)), trace=trace)
    y = np.concatenate([res.results[m]["y"].T for m in range(CORES)], axis=0)
    if trace:
        kernel.last_exec_time_ns = res.exec_time_ns
        kernel.last_results = res
    return y.astype(np.float32)
